# revision 1
# baseline (speedup 1.0000x reference)
"""OCAB (overlapping cross-attention block) Trainium2 Bass kernel.

Full inputs in, full outputs out; internally shards the B*nW window axis
across 8 NeuronCores (each core owns 2 window-rows = 32 image rows, with a
4-row halo for the overlapping k/v windows).

Pipeline per core (all matmuls bf16, fp32 accumulation):
  1. LayerNorm (norm_w/norm_b folded into projection weights on host) +
     PE-transpose to a channels-on-partitions slab with a ones row (biases
     ride the matmuls as an extra input channel).
  2. q/k projections into head-padded transposed slabs (32-row head blocks);
     v projection into a token-major slab (32-col head blocks; cols 30/31 of
     each block are 1.0 so rowsums ride the attention matmul).
  3. Per 16x16 window: S^T = k_patch^T q_patch per head (keys on partitions,
     5 patch-row chunks), exp on ScalarE (PSUM->SBUF bf16), col-packed
     attn@V accumulation, softmax-denominator broadcast via a constant
     matmul + in-place PSUM reciprocal, projection, residual add, DMA out.
"""

import os
import sys
from contextlib import ExitStack

import numpy as np
import ml_dtypes

for _p in ("/opt/trn_rl_repo", "/root/.axon_site/_ro/trn_rl_repo"):
    if os.path.isdir(_p) and _p not in sys.path:
        sys.path.append(_p)

import concourse.bass as bass
import concourse.tile as tile
from concourse import bacc, mybir
from concourse.bass_utils import run_bass_kernel_spmd

BF16 = mybir.dt.bfloat16
F32 = mybir.dt.float32
F32R = mybir.dt.float32r
bfnp = ml_dtypes.bfloat16

# ---- problem constants (hardcoded per contract) ----
C = 180
NH = 6
HD = 30
WS = 16
OWS = 24
PADW = 4
H = W = 256
EPS = 1e-5
NCORES = 8

# ---- per-core slab geometry ----
RS = 40          # slab image rows (32 + 2*4 halo)
CS = 264         # slab image cols (256 + 2*4 zero pad)
TS_REAL = RS * CS          # 10560 real slab tokens
TCH = 84                   # token chunks of 128
T = TCH * 128              # 10752 padded slab tokens
NG = 21                    # 512-token groups (21*512 == 10752)
NWIN = 32                  # windows per core (2 window-rows x 16)
CHUNK_ROWS = [5, 5, 5, 5, 4]       # patch rows per key chunk
CHUNK_KC = [r * OWS for r in CHUNK_ROWS]   # 120,120,120,120,96
# head -> column in the 4-bank S^T psum tile (same-bank pairs share row group)
ST_COL = {0: 0, 4: 256, 1: 512, 5: 768, 2: 1024, 3: 1536}
# head -> column in the packed S_exp sbuf tile
ES_COL = {0: 0, 4: 256, 1: 512, 5: 768, 2: 1024, 3: 1280}

LAST_RESULTS = None
_CACHED = None


def _build_program():
    stage = int(os.environ.get("KSTAGE", "9"))
    nc = bacc.Bacc("TRN2", target_bir_lowering=False)

    xs_d = nc.declare_dram_parameter("xs", [T, C], F32, isOutput=False)
    xr_d = nc.declare_dram_parameter("xr", [8192, C], F32, isOutput=False)
    wqk_d = nc.declare_dram_parameter("wqk", [181, 512], BF16, isOutput=False)
    wv_d = nc.declare_dram_parameter("wv", [181, 192], BF16, isOutput=False)
    wp_d = nc.declare_dram_parameter("wp", [192, C], BF16, isOutput=False)
    id_d = nc.declare_dram_parameter("ident", [128, 128], BF16, isOutput=False)
    e128_d = nc.declare_dram_parameter("e128", [128, 128], F32R, isOutput=False)
    e64_d = nc.declare_dram_parameter("e64", [64, 128], F32R, isOutput=False)
    ones_d = nc.declare_dram_parameter("ones", [1, T], BF16, isOutput=False)
    out_d = nc.declare_dram_parameter("out", [8192, C], F32, isOutput=True)

    with ExitStack() as ctx:
        tc = ctx.enter_context(tile.TileContext(nc))

        # ---- persistent slabs ----
        slab = ctx.enter_context(tc.tile_pool(name="slab", bufs=1))
        XT0 = slab.tile([128, T], BF16, tag="xt0")     # xn^T channels 0-127
        XT1 = slab.tile([53, T], BF16, tag="xt1")      # channels 128-179 + ones row 52
        QS = [
            slab.tile([128, T], BF16, tag=f"qs{i}", name=f"qs{i}") for i in range(4)
        ]
        VS = slab.tile([128, TCH * 192], BF16, tag="vs")

        wp_pool = ctx.enter_context(tc.tile_pool(name="wts", bufs=1))
        WQK0 = wp_pool.tile([128, 512], BF16, tag="wqk0")
        WQK1 = wp_pool.tile([53, 512], BF16, tag="wqk1")
        WV0 = wp_pool.tile([128, 192], BF16, tag="wv0")
        WV1 = wp_pool.tile([53, 192], BF16, tag="wv1")
        WP0 = wp_pool.tile([128, C], BF16, tag="wp0")
        WP1 = wp_pool.tile([64, C], BF16, tag="wp1")
        IDT = wp_pool.tile([128, 128], BF16, tag="id")
        E128 = wp_pool.tile([128, 128], F32R, tag="e128")
        E64 = wp_pool.tile([64, 128], F32R, tag="e64")

        nc.sync.dma_start(WQK0[:], wqk_d[0:128, :])
        nc.sync.dma_start(WQK1[:], wqk_d[128:181, :])
        nc.sync.dma_start(WV0[:], wv_d[0:128, :])
        nc.sync.dma_start(WV1[:], wv_d[128:181, :])
        nc.sync.dma_start(WP0[:], wp_d[0:128, :])
        nc.sync.dma_start(WP1[:], wp_d[128:192, :])
        nc.sync.dma_start(IDT[:], id_d[:, :])
        nc.sync.dma_start(E128[:], e128_d[:, :])
        nc.sync.dma_start(E64[:], e64_d[:, :])
        nc.sync.dma_start(XT1[52:53, :], ones_d[:, :])

        nrep = int(os.environ.get("KREPEAT", "1"))
        for rep in range(nrep):
            # ================= phase 1+2: LN, transpose, projections ============
            with ExitStack() as pctx:
                p_x = pctx.enter_context(tc.tile_pool(name="p_x", bufs=3))
                p_sm = pctx.enter_context(tc.tile_pool(name="p_sm", bufs=4))
                p_ps = pctx.enter_context(tc.tile_pool(name="p_ps", bufs=2, space="PSUM"))

                for g in range(TCH // 4):          # 21 groups of 4 token chunks
                    tp = p_ps.tile([128, 512], BF16, tag="tp")
                    tp2 = p_ps.tile([52, 512], BF16, tag="tp2")
                    for j in range(4):
                        tch = 4 * g + j
                        xt = p_x.tile([128, C], F32, tag="x")
                        nc.sync.dma_start(xt[:], xs_d[128 * tch : 128 * (tch + 1), :])
                        stats = p_sm.tile([128, 6], F32, tag="st")
                        aggr = p_sm.tile([128, 2], F32, tag="ag")
                        nc.vector.bn_stats(stats[:], xt[:])
                        nc.vector.bn_aggr(aggr[:], stats[:])
                        vpe = p_sm.tile([128, 1], F32, tag="vpe")
                        nc.gpsimd.tensor_scalar_add(vpe[:], aggr[:, 1:2], EPS)
                        sd = p_sm.tile([128, 1], F32, tag="sd")
                        nc.scalar.activation(
                            sd[:], vpe[:], mybir.ActivationFunctionType.Sqrt, bias=0.0
                        )
                        rstd = p_sm.tile([128, 1], F32, tag="rstd")
                        nc.vector.reciprocal(rstd[:], sd[:])
                        xn = p_x.tile([128, C], BF16, tag="xn")
                        nc.gpsimd.tensor_scalar(
                            xn[:],
                            xt[:],
                            aggr[:, 0:1],
                            rstd[:],
                            op0=mybir.AluOpType.subtract,
                            op1=mybir.AluOpType.mult,
                        )
                        nc.tensor.transpose(
                            tp[:, 128 * j : 128 * (j + 1)], xn[:, 0:128], IDT[:]
                        )
                        nc.tensor.transpose(
                            tp2[:, 128 * j : 128 * (j + 1)], xn[:, 128:180], IDT[:]
                        )
                    nc.vector.tensor_copy(XT0[:, 512 * g : 512 * (g + 1)], tp[:])
                    nc.vector.tensor_copy(XT1[0:52, 512 * g : 512 * (g + 1)], tp2[:])

                # q^T / k^T projections: 4 M-chunks (q03, q45, k03, k45)
                for mc in range(4):
                    for ng in range(NG):
                        qp = p_ps.tile([128, 512], F32, tag="mm")
                        nc.tensor.matmul(
                            qp[:],
                            WQK0[:, 128 * mc : 128 * (mc + 1)],
                            XT0[:, 512 * ng : 512 * (ng + 1)],
                            start=True,
                            stop=False,
                        )
                        nc.tensor.matmul(
                            qp[:],
                            WQK1[:, 128 * mc : 128 * (mc + 1)],
                            XT1[:, 512 * ng : 512 * (ng + 1)],
                            start=False,
                            stop=True,
                        )
                        nc.vector.tensor_copy(
                            QS[mc][:, 512 * ng : 512 * (ng + 1)], qp[:]
                        )

                # v projection (token-major, 32-col head blocks)
                for pair in range(TCH // 2):
                    vp = p_ps.tile([128, 384], F32, tag="vmm")
                    for j in range(2):
                        tch = 2 * pair + j
                        nc.tensor.matmul(
                            vp[:, 192 * j : 192 * (j + 1)],
                            XT0[:, 128 * tch : 128 * (tch + 1)],
                            WV0[:],
                            start=True,
                            stop=False,
                        )
                        nc.tensor.matmul(
                            vp[:, 192 * j : 192 * (j + 1)],
                            XT1[:, 128 * tch : 128 * (tch + 1)],
                            WV1[:],
                            start=False,
                            stop=True,
                        )
                    nc.vector.tensor_copy(
                        VS[:, 384 * pair : 384 * (pair + 1)], vp[:]
                    )

            # ================= phase 3: windowed attention =======================
            with ExitStack() as actx:
                a_st = actx.enter_context(tc.tile_pool(name="a_st", bufs=1, space="PSUM"))
                a_av = actx.enter_context(tc.tile_pool(name="a_av", bufs=1, space="PSUM"))
                a_ex = actx.enter_context(tc.tile_pool(name="a_ex", bufs=1, space="PSUM"))
                a_es = actx.enter_context(tc.tile_pool(name="a_es", bufs=6))
                a_vw = actx.enter_context(tc.tile_pool(name="a_vw", bufs=12))
                a_sb = actx.enter_context(tc.tile_pool(name="a_sb", bufs=2))

                qs_pat = [
                    QS[i][:, 0:TS_REAL].rearrange("p (r c) -> p r c", c=CS)
                    for i in range(4)
                ]

                if stage == 0:
                    # debug: dump VS slab into out
                    dbg = a_sb.tile([128, 360], F32, tag="ot", name=f"dbg0_{rep}")
                    nc.vector.tensor_copy(dbg[:, 0:180], VS[:, 0:180])
                    nc.sync.dma_start(out_d[0:128, :], dbg[:, 0:180])
                for w in range(NWIN if stage >= 1 else 0):
                    wrl, wc = w // 16, w % 16
                    r0, c0 = WS * wrl, WS * wc

                    # ---- gather v windows (SBUF->SBUF DMA, per patch row) ----
                    vw = [
                        a_vw.tile([128, 192], BF16, tag="vw", name=f"vw{rep}_{w}_{i}")
                        for i in range(5)
                    ]
                    for pr in range(OWS):
                        ch, rr = pr // 5, pr % 5
                        t0 = CS * (r0 + pr) + c0
                        done = 0
                        while done < OWS:
                            p0 = (t0 + done) % 128
                            blk = (t0 + done) // 128
                            n = min(OWS - done, 128 - p0)
                            nc.sync.dma_start(
                                vw[ch][OWS * rr + done : OWS * rr + done + n, :],
                                VS[p0 : p0 + n, 192 * blk : 192 * (blk + 1)],
                            )
                            done += n

                    av = a_av.tile([128, 512], F32, tag="av")

                    # materialize k^T windows (stationary matmul APs must be
                    # single-free-dim; moving APs may stay 3D)
                    kw0 = a_vw.tile([128, 576], BF16, tag="kw0", name=f"kw0_{rep}_{w}", bufs=2)
                    kw1 = a_vw.tile([64, 576], BF16, tag="kw1", name=f"kw1_{rep}_{w}", bufs=2)
                    nc.vector.tensor_copy(
                        kw0[:].rearrange("p (r c) -> p r c", c=OWS),
                        qs_pat[2][:, r0 : r0 + OWS, c0 : c0 + OWS],
                    )
                    nc.vector.tensor_copy(
                        kw1[:].rearrange("p (r c) -> p r c", c=OWS),
                        qs_pat[3][0:64, r0 : r0 + OWS, c0 : c0 + OWS],
                    )

                    if stage == 1:
                        dbg1 = a_sb.tile([128, 360], F32, tag="ot", name=f"dbg1_{rep}_{w}")
                        nc.vector.tensor_copy(dbg1[:, 0:192], vw[0][:, :])
                        nc.vector.tensor_copy(dbg1[:, 192:336], kw0[:, 0:144])
                        nc.sync.dma_start(
                            out_d[256 * w : 256 * w + 128, 0:180], dbg1[:, 0:180]
                        )
                        continue

                    if stage == 21:
                        heads = [0]
                    elif stage == 25:
                        heads = [1]
                    elif stage == 26:
                        heads = [0, 1, 2, 3]
                    elif stage == 27:
                        heads = [0, 1]
                    elif stage == 28:
                        heads = [0, 4]
                    elif stage == 22:
                        heads = [0, 1, 2]
                    elif stage == 23:
                        heads = [0, 1, 2, 3]
                    elif stage == 24:
                        heads = [0, 1, 2, 4, 5]
                    else:
                        heads = list(range(NH))
                    do_exp = stage not in (21, 22, 23, 24, 25, 26, 27, 28)
                    kdim = 32 if stage in (26,) or stage >= 9 else HD

                    es_list = []
                    for ch in range(5):
                        kc = CHUNK_KC[ch]
                        # Concurrent row-group-packed matmuls must write distinct
                        # PSUM banks: st is 4 banks; same-bank head pairs share a
                        # row group so the sub-array serializes them.
                        st = a_st.tile([128, 2048], F32, tag="st", name=f"st{rep}_{w}_{ch}")
                        for h in heads:
                            ktile = kw0 if h < 4 else kw1
                            qtile = qs_pat[0] if h < 4 else qs_pat[1]
                            hr = 32 * (h % 4)
                            kpat = ktile[hr : hr + kdim, 120 * ch : 120 * ch + kc]
                            qpat = qtile[
                                hr : hr + kdim,
                                PADW + WS * wrl : PADW + WS * wrl + WS,
                                PADW + c0 : PADW + c0 + WS,
                            ]
                            nc.tensor.matmul(
                                st[0:kc, ST_COL[h] : ST_COL[h] + 256],
                                kpat,
                                qpat,
                                start=True,
                                stop=True,
                                tile_position=(hr, 0),
                            )
                        es = a_es.tile([128, 1536], BF16, tag="es", name=f"es{rep}_{w}_{ch}")
                        if do_exp:
                            nc.scalar.activation(
                                es[0:kc, 0:1024],
                                st[0:kc, 0:1024],
                                mybir.ActivationFunctionType.Exp,
                            )
                            nc.scalar.activation(
                                es[0:kc, 1024:1536].rearrange(
                                    "p (a b) -> p a b", b=256
                                ),
                                st[0:kc, 1024:2048].rearrange("p (a b) -> p a b", b=512)[
                                    :, :, 0:256
                                ],
                                mybir.ActivationFunctionType.Exp,
                            )
                        else:
                            nc.vector.tensor_copy(
                                es[0:kc, 0:256], st[0:kc, 0:256]
                            )
                        es_list.append(es)

                    if stage in (2, 21, 22, 23, 24, 25, 26, 27, 28):
                        dbg2 = a_sb.tile([128, 360], F32, tag="ot", name=f"dbg2_{rep}_{w}")
                        nc.vector.tensor_copy(dbg2[:, 0:256], es_list[0][:, 0:256])
                        nc.sync.dma_start(
                            out_d[256 * w : 256 * w + 128, 0:180], dbg2[:, 0:180]
                        )
                        continue

                    # ---- attn @ V (col-packed; h4/h5 duplicated to fill psum) ----
                    # head-major so psum accumulation groups are sequential
                    av_jobs = [(h, 32 * (h % 4), 256 * (h // 4)) for h in range(NH)]
                    av_jobs += [(h, 64 + 32 * (h - 4), 256) for h in (4, 5)]
                    for h, colp, colf in av_jobs:
                        for ch in range(5):
                            kc = CHUNK_KC[ch]
                            nc.tensor.matmul(
                                av[colp : colp + 32, colf : colf + 256],
                                vw[ch][0:kc, 32 * h : 32 * h + 32],
                                es_list[ch][0:kc, ES_COL[h] : ES_COL[h] + 256],
                                start=(ch == 0),
                                stop=(ch == 4),
                                tile_position=(0, colp),
                            )

                    if stage == 3:
                        dbg3 = a_sb.tile([128, 360], F32, tag="ot", name=f"dbg3_{rep}_{w}")
                        nc.vector.tensor_copy(dbg3[:, 0:256], av[:, 0:256])
                        nc.sync.dma_start(
                            out_d[256 * w : 256 * w + 128, 0:180], dbg3[:, 0:180]
                        )
                        continue

                    # ---- softmax normalize + project + residual ----
                    rsb = a_sb.tile([128, 512], F32R, tag="rsb")
                    nc.vector.tensor_copy(rsb[:], av[:])
                    ex = a_ex.tile([128, 512], F32, tag="ex")
                    nc.tensor.matmul(
                        ex[:, 0:256],
                        E128[:],
                        rsb[:, 0:256],
                        start=True,
                        stop=True,
                    )
                    nc.tensor.matmul(
                        ex[:, 256:512],
                        E64[:],
                        rsb[0:64, 256:512],
                        start=True,
                        stop=True,
                    )
                    ex_sb = a_sb.tile([128, 512], F32, tag="exsb")
                    nc.vector.reciprocal(ex_sb[:], ex[:])
                    att = a_sb.tile([128, 512], BF16, tag="att")
                    nc.vector.tensor_tensor(
                        att[:], av[:], ex_sb[:], op=mybir.AluOpType.mult
                    )

                    pp = a_ex.tile([128, 360], F32, tag="pp", name=f"pp{rep}_{w}")
                    for qc in range(2):
                        nc.tensor.matmul(
                            pp[:, 180 * qc : 180 * qc + 180],
                            att[:, 128 * qc : 128 * (qc + 1)],
                            WP0[:],
                            start=True,
                            stop=False,
                        )
                        nc.tensor.matmul(
                            pp[:, 180 * qc : 180 * qc + 180],
                            att[0:64, 256 + 128 * qc : 256 + 128 * (qc + 1)],
                            WP1[:],
                            start=False,
                            stop=True,
                        )
                    xres = a_sb.tile([128, 360], F32, tag="xres")
                    ot = a_sb.tile([128, 360], F32, tag="ot")
                    xr_pat = xr_d[:, :].rearrange("(r c) d -> r c d", c=W)
                    for qc in range(2):
                        rq = WS * wrl + 8 * qc
                        nc.sync.dma_start(
                            xres[:, 180 * qc : 180 * qc + 180],
                            xr_pat[rq : rq + 8, c0 : c0 + WS, :],
                        )
                    nc.vector.tensor_tensor(
                        ot[:], pp[:], xres[:], op=mybir.AluOpType.add
                    )
                    out_pat = out_d[:, :].rearrange("(r c) d -> r c d", c=W)
                    for qc in range(2):
                        rq = WS * wrl + 8 * qc
                        nc.sync.dma_start(
                            out_pat[rq : rq + 8, c0 : c0 + WS, :],
                            ot[:, 180 * qc : 180 * qc + 180],
                        )

    nc.compile()
    return nc


def _prep_host(inputs):
    x = np.ascontiguousarray(inputs["x"], dtype=np.float32)[0]  # [65536, 180]
    norm_w = np.asarray(inputs["norm_w"], np.float32)
    norm_b = np.asarray(inputs["norm_b"], np.float32)
    q_w = np.asarray(inputs["q_w"], np.float32)
    q_b = np.asarray(inputs["q_b"], np.float32)
    kv_w = np.asarray(inputs["kv_w"], np.float32)
    kv_b = np.asarray(inputs["kv_b"], np.float32)
    proj_w = np.asarray(inputs["proj_w"], np.float32)
    proj_b = np.asarray(inputs["proj_b"], np.float32)

    scale = HD ** -0.5
    Wq = norm_w[:, None] * q_w * scale
    bq = (norm_b @ q_w + q_b) * scale
    Wk = norm_w[:, None] * kv_w[:, :C]
    bk = norm_b @ kv_w[:, :C] + kv_b[:C]
    Wv = norm_w[:, None] * kv_w[:, C:]
    bv = norm_b @ kv_w[:, C:] + kv_b[C:]

    # wqk [181, 512]: 4 M-chunks (q h0-3 | q h4-5 | k h0-3 | k h4-5), 32-col head blocks
    wqk = np.zeros((181, 512), np.float32)
    for h in range(NH):
        mc = 0 if h < 4 else 1
        col = 128 * mc + 32 * (h % 4)
        wqk[:C, col : col + HD] = Wq[:, HD * h : HD * (h + 1)]
        wqk[C, col : col + HD] = bq[HD * h : HD * (h + 1)]
        colk = 256 + col
        wqk[:C, colk : colk + HD] = Wk[:, HD * h : HD * (h + 1)]
        wqk[C, colk : colk + HD] = bk[HD * h : HD * (h + 1)]

    # wv [181, 192]: 32-col head blocks; cols 30/31 of each block = ones (bias row)
    wv = np.zeros((181, 192), np.float32)
    for h in range(NH):
        col = 32 * h
        wv[:C, col : col + HD] = Wv[:, HD * h : HD * (h + 1)]
        wv[C, col : col + HD] = bv[HD * h : HD * (h + 1)]
        wv[C, col + 30] = 1.0
        wv[C, col + 31] = 1.0

    # wp [192, 180]: head-padded proj rows
    wp = np.zeros((192, C), np.float32)
    for h in range(NH):
        row = 32 * (h % 4) if h < 4 else 128 + 32 * (h - 4)
        wp[row : row + HD, :] = proj_w[HD * h : HD * (h + 1), :]

    ident = np.eye(128, dtype=bfnp)
    e128 = np.zeros((128, 128), np.float32)
    for j in range(4):
        e128[32 * j + 30, 32 * j : 32 * j + 32] = 1.0
    e64 = np.zeros((64, 128), np.float32)
    for j in range(2):
        e64[32 * j + 30, 32 * j : 32 * j + 32] = 1.0
        e64[32 * j + 30, 64 + 32 * j : 64 + 32 * j + 32] = 1.0

    # per-core slabs
    xg = x.reshape(H, W, C)
    xpad = np.zeros((H + 2 * PADW, CS, C), np.float32)
    xpad[PADW : PADW + H, PADW : PADW + W, :] = xg
    xres_full = x + proj_b  # residual (+proj bias folded in)

    in_maps = []
    for c in range(NCORES):
        slab = np.zeros((T, C), np.float32)
        slab[:TS_REAL] = xpad[32 * c : 32 * c + RS].reshape(TS_REAL, C)
        xr = np.ascontiguousarray(
            xres_full[8192 * c : 8192 * (c + 1)], dtype=np.float32
        )
        in_maps.append(
            {
                "xs": slab,
                "xr": xr,
                "wqk": wqk.astype(bfnp),
                "wv": wv.astype(bfnp),
                "wp": wp.astype(bfnp),
                "ident": ident,
                "e128": e128,
                "e64": e64,
                "ones": np.ones((1, T), bfnp),
            }
        )
    return in_maps


def kernel(**inputs):
    global _CACHED, LAST_RESULTS
    if _CACHED is None:
        _CACHED = _build_program()
    nc = _CACHED
    in_maps = _prep_host(inputs)
    res = run_bass_kernel_spmd(
        nc,
        in_maps,
        list(range(NCORES)),
        trace=False,
    )
    LAST_RESULTS = res
    out = np.empty((1, H * W, C), np.float32)
    for c in range(NCORES):
        out[0, 8192 * c : 8192 * (c + 1), :] = res.results[c]["out"]
    return out



# revision 26
# speedup vs baseline: 1.4154x; 1.4154x over previous
"""OCAB (overlapping cross-attention block) Trainium2 Bass kernel, v2.

Full inputs in, full outputs out; internally shards the B*nW window axis
across 8 NeuronCores (each core owns 2 window-rows = 32 image rows, with a
4-row halo for the overlapping k/v windows).

v2 layout changes vs v1 (which was PE-bound at isolated-matmul latency and
Sync-bound on ~29 tiny SBUF->SBUF v-gather DMAs per window):
  - V slab stored column-major in 3 overlapping 128-col blocks
    (slab cols 0-127 / 112-239 / 136-263) so every 24-col window slice
    lives in one block at a contiguous partition range; the per-window
    v-gather is 5 DMAs (one per 5-row key chunk), each [24p, rn*192]
    contiguous-source -> [kc, 192] key-major tile.
  - Window key order is column-major within a chunk (key = rn*cl + rl),
    consistently in the k^T chunk tiles and the v tiles.
  - S^T PSUM is one 3-bank [kc, 1536] tile (head col pairs share banks
    only when they share a PE row group, except h2/h3 which Tile
    serializes); exp is a single ACT instruction per chunk.
  - attn@V runs chunk-major so PE accumulates av(ch) while ACT computes
    exp(ch+1); S^T(ch+1) follows once the st bank frees.
  - x loads batched 4 token-chunks per DMA; LN rstd via Sqrt(bias=eps) +
    reciprocal_approx_fast; normalize split across DVE and GpSimd.
"""

import os
import sys
from contextlib import ExitStack

import numpy as np
import ml_dtypes

for _p in ("/opt/trn_rl_repo", "/root/.axon_site/_ro/trn_rl_repo"):
    if os.path.isdir(_p) and _p not in sys.path:
        sys.path.append(_p)

import concourse.bass as bass
import concourse.tile as tile
from concourse import bacc, mybir
from concourse.bass_utils import run_bass_kernel_spmd

BF16 = mybir.dt.bfloat16
F32 = mybir.dt.float32
F32R = mybir.dt.float32r
bfnp = ml_dtypes.bfloat16

# ---- problem constants (hardcoded per contract) ----
C = 180
NH = 6
HD = 30
WS = 16
OWS = 24
PADW = 4
H = W = 256
EPS = 1e-5
NCORES = 8

# ---- per-core slab geometry ----
RS = 40          # slab image rows (32 + 2*4 halo)
CS = 264         # slab image cols (256 + 2*4 zero pad)
TS_REAL = RS * CS          # 10560 real slab tokens
TCH = 84                   # token chunks of 128
T = TCH * 128              # 10752 padded slab tokens
NG = 21                    # 512-token groups
NWIN = 32                  # windows per core (2 window-rows x 16)
CHUNK_ROWS = [5, 5, 5, 5, 4]
CHUNK_KC = [r * OWS for r in CHUNK_ROWS]   # 120,120,120,120,96
# head -> column in both the 3-bank S^T psum tile and the es sbuf tile.
# Heads sharing a psum bank MUST share a PE row group (the sub-array then
# serializes their writes; different row groups would write one bank
# concurrently = HW error). h3 therefore lives in the q45/k45 slabs at
# rows 64-95: bank pairs (h0,h4)@rows0, (h1,h5)@rows32, (h2,h3)@rows64.
HCOL = {0: 0, 4: 256, 1: 512, 5: 768, 2: 1024, 3: 1280}
SROW = {0: 0, 1: 32, 2: 64, 3: 64, 4: 0, 5: 32}   # PE row group per head
SORDER = [0, 1, 2, 4, 5, 3]
# v slab column blocks: block cg holds slab cols VCG_BASE[cg]..+127,
# partition = col - VCG_BASE[cg]; window wc reads block WC_CG[wc].
VCG_BASE = [0, 112, 136]
WC_CG = [0] * 7 + [1] * 7 + [2] * 2
VS_COLS = 3 * RS * 192     # cg-major block layout: block = 40*cg + r

LAST_RESULTS = None
_CACHED = None


def _build_program():
    nc = bacc.Bacc("TRN2", target_bir_lowering=False)

    xs_d = nc.declare_dram_parameter("xs", [T, C], F32, isOutput=False)
    xr_d = nc.declare_dram_parameter("xr", [8192, C], F32, isOutput=False)
    wqk_d = nc.declare_dram_parameter("wqk", [181, 512], BF16, isOutput=False)
    wv_d = nc.declare_dram_parameter("wv", [181, 192], BF16, isOutput=False)
    wp_d = nc.declare_dram_parameter("wp", [192, C], BF16, isOutput=False)
    id_d = nc.declare_dram_parameter("ident", [128, 128], BF16, isOutput=False)
    e128_d = nc.declare_dram_parameter("e128", [128, 128], F32R, isOutput=False)
    e64_d = nc.declare_dram_parameter("e64", [64, 128], F32R, isOutput=False)
    ones_d = nc.declare_dram_parameter("ones", [1, T], BF16, isOutput=False)
    out_d = nc.declare_dram_parameter("out", [8192, C], F32, isOutput=True)

    with ExitStack() as ctx:
        tc = ctx.enter_context(tile.TileContext(nc))

        # ---- persistent slabs (live through both phases) ----
        keep = ctx.enter_context(tc.tile_pool(name="keep", bufs=1))
        QS = [
            keep.tile([128, T], BF16, tag=f"qs{i}", name=f"qs{i}") for i in range(4)
        ]
        VS = keep.tile([128, VS_COLS], BF16, tag="vs")

        WQK0 = keep.tile([128, 512], BF16, tag="wqk0")
        WQK1 = keep.tile([53, 512], BF16, tag="wqk1")
        WV0 = keep.tile([128, 192], BF16, tag="wv0")
        WV1 = keep.tile([53, 192], BF16, tag="wv1")
        WP0 = keep.tile([128, C], BF16, tag="wp0")
        WP1 = keep.tile([64, C], BF16, tag="wp1")
        IDT = keep.tile([128, 128], BF16, tag="id")
        E128 = keep.tile([128, 128], F32R, tag="e128")
        E64 = keep.tile([64, 128], F32R, tag="e64")
        EPST = keep.tile([128, 1], F32, tag="epst")
        nc.vector.memset(EPST[:], EPS)

        nc.sync.dma_start(WQK0[:], wqk_d[0:128, :])
        nc.sync.dma_start(WQK1[:], wqk_d[128:181, :])
        nc.sync.dma_start(WV0[:], wv_d[0:128, :])
        nc.sync.dma_start(WV1[:], wv_d[128:181, :])
        nc.sync.dma_start(WP0[:], wp_d[0:128, :])
        nc.sync.dma_start(WP1[:], wp_d[128:192, :])
        nc.sync.dma_start(IDT[:], id_d[:, :])
        nc.sync.dma_start(E128[:], e128_d[:, :])
        nc.sync.dma_start(E64[:], e64_d[:, :])

        # ================= phase 1+2: LN, transpose, projections ============
        with ExitStack() as pctx:
            ph2 = pctx.enter_context(tc.tile_pool(name="ph2", bufs=1))
            XT0 = ph2.tile([128, T], BF16, tag="xt0")     # xn^T channels 0-127
            XT1 = ph2.tile([53, T], BF16, tag="xt1")      # ch 128-179 + ones row
            nc.sync.dma_start(XT1[52:53, :], ones_d[:, :])

            p_x = pctx.enter_context(tc.tile_pool(name="p_x", bufs=3))
            p_sm = pctx.enter_context(tc.tile_pool(name="p_sm", bufs=8))
            p_ps = pctx.enter_context(tc.tile_pool(name="p_ps", bufs=2, space="PSUM"))

            for g in range(NG):            # 21 groups of 4 token chunks
                xt4 = p_x.tile([128, 720], F32, tag="x")
                nc.sync.dma_start(
                    xt4[:],
                    xs_d[512 * g : 512 * (g + 1), :]
                    .rearrange("(j p) c -> j p c", p=128)
                    .transpose([1, 0, 2]),
                )
                tp = p_ps.tile([128, 512], BF16, tag="tp")
                tp2 = p_ps.tile([52, 512], BF16, tag="tp2")
                for j in range(4):
                    xsl = xt4[:, 180 * j : 180 * (j + 1)]
                    stats = p_sm.tile([128, 6], F32, tag="st")
                    aggr = p_sm.tile([128, 2], F32, tag="ag")
                    nc.vector.bn_stats(stats[:], xsl)
                    nc.vector.bn_aggr(aggr[:], stats[:])
                    sd = p_sm.tile([128, 1], F32, tag="sd")
                    nc.scalar.activation(
                        sd[:], aggr[:, 1:2], mybir.ActivationFunctionType.Sqrt,
                        bias=EPST[:],
                    )
                    rstd = p_sm.tile([128, 1], F32, tag="rstd")
                    if os.environ.get("KNOAPPROX", "0") == "1":
                        nc.vector.reciprocal(rstd[:], sd[:])
                    else:
                        nc.vector.reciprocal_approx_fast(rstd[:], sd[:])
                    xn = p_x.tile([128, C], BF16, tag="xn")
                    if os.environ.get("KNOVTS", "0") == "1":
                        eng = nc.gpsimd
                    else:
                        eng = nc.vector if j % 2 == 0 else nc.gpsimd
                    eng.tensor_scalar(
                        xn[:],
                        xsl,
                        aggr[:, 0:1],
                        rstd[:],
                        op0=mybir.AluOpType.subtract,
                        op1=mybir.AluOpType.mult,
                    )
                    nc.tensor.transpose(
                        tp[:, 128 * j : 128 * (j + 1)], xn[:, 0:128], IDT[:]
                    )
                    nc.tensor.transpose(
                        tp2[:, 128 * j : 128 * (j + 1)], xn[:, 128:180], IDT[:]
                    )
                nc.vector.tensor_copy(XT0[:, 512 * g : 512 * (g + 1)], tp[:])
                nc.vector.tensor_copy(XT1[0:52, 512 * g : 512 * (g + 1)], tp2[:])

            # q^T / k^T projections: 4 M-chunks (q03, q45, k03, k45)
            for mc in range(4):
                for ng in range(NG):
                    qp = p_ps.tile([128, 512], F32, tag="mm")
                    nc.tensor.matmul(
                        qp[:],
                        WQK0[:, 128 * mc : 128 * (mc + 1)],
                        XT0[:, 512 * ng : 512 * (ng + 1)],
                        start=True,
                        stop=False,
                    )
                    nc.tensor.matmul(
                        qp[:],
                        WQK1[:, 128 * mc : 128 * (mc + 1)],
                        XT1[:, 512 * ng : 512 * (ng + 1)],
                        start=False,
                        stop=True,
                    )
                    nc.vector.tensor_copy(
                        QS[mc][:, 512 * ng : 512 * (ng + 1)], qp[:]
                    )

            # v projection into column-major blocks: block b = 40*cg + r holds
            # v for slab cols VCG_BASE[cg]..+127 of image row r (col on
            # partitions), two blocks per psum tile.
            for i in range(60):
                vp = p_ps.tile([128, 384], F32, tag="vp")
                for j in range(2):
                    b = 2 * i + j
                    cg, r = divmod(b, RS)
                    tok0 = CS * r + VCG_BASE[cg]
                    nc.tensor.matmul(
                        vp[:, 192 * j : 192 * (j + 1)],
                        XT0[:, tok0 : tok0 + 128],
                        WV0[:],
                        start=True,
                        stop=False,
                    )
                    nc.tensor.matmul(
                        vp[:, 192 * j : 192 * (j + 1)],
                        XT1[:, tok0 : tok0 + 128],
                        WV1[:],
                        start=False,
                        stop=True,
                    )
                nc.vector.tensor_copy(VS[:, 384 * i : 384 * (i + 1)], vp[:])

        # ================= phase 3: windowed attention =======================
        with ExitStack() as actx:
            a_st = actx.enter_context(tc.tile_pool(name="a_st", bufs=1, space="PSUM"))
            a_av = actx.enter_context(tc.tile_pool(name="a_av", bufs=3, space="PSUM"))
            a_ep = actx.enter_context(tc.tile_pool(name="a_ep", bufs=2, space="PSUM"))
            a_es = actx.enter_context(tc.tile_pool(name="a_es", bufs=7))
            a_kv = actx.enter_context(tc.tile_pool(name="a_kv", bufs=12))
            a_sb = actx.enter_context(tc.tile_pool(name="a_sb", bufs=2))

            qs_pat = [
                QS[i][:, 0:TS_REAL].rearrange("p (r c) -> p r c", c=CS)
                for i in range(4)
            ]
            # head-major av jobs (h4/h5 duplicated to fill psum col groups)
            av_jobs = [(h, 32 * (h % 4), 256 * (h // 4)) for h in range(NH)]
            av_jobs += [(h, 64 + 32 * (h - 4), 256) for h in (4, 5)]

            stage = int(os.environ.get("KSTAGE", "9"))
            nwin = NWIN if stage >= 1 else 0
            for w in range(nwin):
                wrl, wc = w // 16, w % 16
                r0, c0 = WS * wrl, WS * wc
                cg = WC_CG[wc]
                pbase = c0 - VCG_BASE[cg]

                # ---- per-chunk v gather + k^T chunk tiles (keys row-major:
                # key = 24*rl + cl). DMA partition dims must MATCH on both
                # sides (mismatched counts wedge the DGE), so gather one slab
                # row per DMA: [24p, 192] -> [24p, 192], no boundary splits
                # thanks to the column-block VS layout.
                vw, kw0c, kw1c = [], [], []
                for ch in range(5):
                    rn = CHUNK_ROWS[ch]
                    kc = CHUNK_KC[ch]
                    R0 = r0 + 5 * ch
                    v = a_kv.tile([128, 192], BF16, tag="vw", name=f"vw{w}_{ch}")
                    for rl in range(rn):
                        blk = (RS * cg + R0 + rl) * 192
                        eng = nc.sync if (ch + rl) % 2 == 0 else nc.gpsimd
                        eng.dma_start(
                            v[24 * rl : 24 * rl + 24, :],
                            VS[pbase : pbase + 24, blk : blk + 192],
                        )
                    vw.append(v)
                    k0 = a_kv.tile([128, 120], BF16, tag="kw0", name=f"k0_{w}_{ch}")
                    nc.vector.tensor_copy(
                        k0[:, 0:kc].rearrange("p (rl cl) -> p rl cl", cl=24),
                        qs_pat[2][:, R0 : R0 + rn, c0 : c0 + 24],
                    )
                    kw0c.append(k0)
                    k1 = a_kv.tile([96, 120], BF16, tag="kw1", name=f"k1_{w}_{ch}")
                    nc.vector.tensor_copy(
                        k1[:, 0:kc].rearrange("p (rl cl) -> p rl cl", cl=24),
                        qs_pat[3][0:96, R0 : R0 + rn, c0 : c0 + 24],
                    )
                    kw1c.append(k1)

                if stage < 2:
                    continue

                es_list = []
                for ch in range(5):
                    kc = CHUNK_KC[ch]
                    st = a_st.tile([128, 1536], F32, tag="st", name=f"st{w}_{ch}")
                    for h in SORDER:
                        hr = SROW[h]
                        ktile = kw0c[ch] if h < 3 else kw1c[ch]
                        qtile = qs_pat[0] if h < 3 else qs_pat[1]
                        nc.tensor.matmul(
                            st[0:kc, HCOL[h] : HCOL[h] + 256],
                            ktile[hr : hr + 32, 0:kc],
                            qtile[
                                hr : hr + 32,
                                PADW + r0 : PADW + r0 + WS,
                                PADW + c0 : PADW + c0 + WS,
                            ],
                            start=True,
                            stop=True,
                            tile_position=(hr, 0),
                        )
                    es = a_es.tile([128, 1536], BF16, tag="es", name=f"es{w}_{ch}")
                    nc.scalar.activation(
                        es[0:kc, :], st[0:kc, :], mybir.ActivationFunctionType.Exp
                    )
                    es_list.append(es)

                if stage < 3:
                    continue
                av = a_av.tile([128, 512], F32, tag="av", name=f"av{w}")
                # attn@V, job-major (accumulation groups per psum partition
                # range must be sequential); overlaps with the NEXT window's
                # S^T/exp via pool double-buffering.
                for h, colp, colf in av_jobs:
                    for ch in range(5):
                        kc = CHUNK_KC[ch]
                        nc.tensor.matmul(
                            av[colp : colp + 32, colf : colf + 256],
                            vw[ch][0:kc, 32 * h : 32 * h + 32],
                            es_list[ch][0:kc, HCOL[h] : HCOL[h] + 256],
                            start=(ch == 0),
                            stop=(ch == 4),
                            tile_position=(0, colp),
                        )

                if stage < 4:
                    continue
                # ---- softmax normalize + project + residual ----
                rsb = a_sb.tile([128, 512], F32R, tag="rsb")
                nc.vector.tensor_copy(rsb[:], av[:])
                ex = a_ep.tile([128, 512], F32, tag="ep", name=f"ex{w}")
                nc.tensor.matmul(
                    ex[:, 0:256], E128[:], rsb[:, 0:256], start=True, stop=True
                )
                nc.tensor.matmul(
                    ex[:, 256:512], E64[:], rsb[0:64, 256:512], start=True, stop=True
                )
                ex_sb = a_sb.tile([128, 512], F32, tag="exsb")
                if os.environ.get("KNOAPPROX", "0") == "1":
                    nc.vector.reciprocal(ex_sb[:], ex[:])
                else:
                    nc.vector.reciprocal_approx_fast(ex_sb[:], ex[:])
                att = a_sb.tile([128, 512], BF16, tag="att")
                nc.vector.tensor_tensor(
                    att[:], av[:], ex_sb[:], op=mybir.AluOpType.mult
                )

                pp = a_ep.tile([128, 360], F32, tag="ep", name=f"pp{w}")
                for qc in range(2):
                    nc.tensor.matmul(
                        pp[:, 180 * qc : 180 * qc + 180],
                        att[:, 128 * qc : 128 * (qc + 1)],
                        WP0[:],
                        start=True,
                        stop=False,
                    )
                    nc.tensor.matmul(
                        pp[:, 180 * qc : 180 * qc + 180],
                        att[0:64, 256 + 128 * qc : 256 + 128 * (qc + 1)],
                        WP1[:],
                        start=False,
                        stop=True,
                    )
                xres = a_sb.tile([128, 360], F32, tag="xres")
                ot = a_sb.tile([128, 360], F32, tag="ot")
                xr_pat = xr_d[:, :].rearrange("(r c) d -> r c d", c=W)
                for qc in range(2):
                    rq = WS * wrl + 8 * qc
                    nc.sync.dma_start(
                        xres[:, 180 * qc : 180 * qc + 180],
                        xr_pat[rq : rq + 8, c0 : c0 + WS, :],
                    )
                nc.vector.tensor_tensor(
                    ot[:], pp[:], xres[:], op=mybir.AluOpType.add
                )
                out_pat = out_d[:, :].rearrange("(r c) d -> r c d", c=W)
                for qc in range(2):
                    rq = WS * wrl + 8 * qc
                    nc.sync.dma_start(
                        out_pat[rq : rq + 8, c0 : c0 + WS, :],
                        ot[:, 180 * qc : 180 * qc + 180],
                    )

    nc.compile()
    return nc


def _prep_host(inputs):
    x = np.ascontiguousarray(inputs["x"], dtype=np.float32)[0]  # [65536, 180]
    norm_w = np.asarray(inputs["norm_w"], np.float32)
    norm_b = np.asarray(inputs["norm_b"], np.float32)
    q_w = np.asarray(inputs["q_w"], np.float32)
    q_b = np.asarray(inputs["q_b"], np.float32)
    kv_w = np.asarray(inputs["kv_w"], np.float32)
    kv_b = np.asarray(inputs["kv_b"], np.float32)
    proj_w = np.asarray(inputs["proj_w"], np.float32)
    proj_b = np.asarray(inputs["proj_b"], np.float32)

    scale = HD ** -0.5
    Wq = norm_w[:, None] * q_w * scale
    bq = (norm_b @ q_w + q_b) * scale
    Wk = norm_w[:, None] * kv_w[:, :C]
    bk = norm_b @ kv_w[:, :C] + kv_b[:C]
    Wv = norm_w[:, None] * kv_w[:, C:]
    bv = norm_b @ kv_w[:, C:] + kv_b[C:]

    # wqk [181, 512]: 4 M-chunks (q h0-2 | q h4,h5,h3 | k h0-2 | k h4,h5,h3)
    # h3 rides the second chunk at rows 64-95 so its psum-bank partner h2
    # shares PE row group 64 (see SROW in _build_program).
    wqk = np.zeros((181, 512), np.float32)
    for h in range(NH):
        if h < 3:
            col = 32 * h
        else:
            col = 128 + {3: 64, 4: 0, 5: 32}[h]
        wqk[:C, col : col + HD] = Wq[:, HD * h : HD * (h + 1)]
        wqk[C, col : col + HD] = bq[HD * h : HD * (h + 1)]
        colk = 256 + col
        wqk[:C, colk : colk + HD] = Wk[:, HD * h : HD * (h + 1)]
        wqk[C, colk : colk + HD] = bk[HD * h : HD * (h + 1)]

    # wv [181, 192]: 32-col head blocks; cols 30/31 of each block = ones
    wv = np.zeros((181, 192), np.float32)
    for h in range(NH):
        col = 32 * h
        wv[:C, col : col + HD] = Wv[:, HD * h : HD * (h + 1)]
        wv[C, col : col + HD] = bv[HD * h : HD * (h + 1)]
        wv[C, col + 30] = 1.0
        wv[C, col + 31] = 1.0

    # wp [192, 180]: head-padded proj rows
    wp = np.zeros((192, C), np.float32)
    for h in range(NH):
        row = 32 * (h % 4) if h < 4 else 128 + 32 * (h - 4)
        wp[row : row + HD, :] = proj_w[HD * h : HD * (h + 1), :]

    ident = np.eye(128, dtype=bfnp)
    e128 = np.zeros((128, 128), np.float32)
    for j in range(4):
        e128[32 * j + 30, 32 * j : 32 * j + 32] = 1.0
    e64 = np.zeros((64, 128), np.float32)
    for j in range(2):
        e64[32 * j + 30, 32 * j : 32 * j + 32] = 1.0
        e64[32 * j + 30, 64 + 32 * j : 64 + 32 * j + 32] = 1.0

    # per-core slabs
    xg = x.reshape(H, W, C)
    xpad = np.zeros((H + 2 * PADW, CS, C), np.float32)
    xpad[PADW : PADW + H, PADW : PADW + W, :] = xg
    xres_full = x + proj_b

    in_maps = []
    for c in range(NCORES):
        slab = np.zeros((T, C), np.float32)
        slab[:TS_REAL] = xpad[32 * c : 32 * c + RS].reshape(TS_REAL, C)
        xr = np.ascontiguousarray(
            xres_full[8192 * c : 8192 * (c + 1)], dtype=np.float32
        )
        in_maps.append(
            {
                "xs": slab,
                "xr": xr,
                "wqk": wqk.astype(bfnp),
                "wv": wv.astype(bfnp),
                "wp": wp.astype(bfnp),
                "ident": ident,
                "e128": e128,
                "e64": e64,
                "ones": np.ones((1, T), bfnp),
            }
        )
    return in_maps


def kernel(**inputs):
    global _CACHED, LAST_RESULTS
    if _CACHED is None:
        _CACHED = _build_program()
    nc = _CACHED
    in_maps = _prep_host(inputs)
    res = run_bass_kernel_spmd(
        nc,
        in_maps,
        list(range(NCORES)),
        trace=False,
    )
    LAST_RESULTS = res
    out = np.empty((1, H * W, C), np.float32)
    for c in range(NCORES):
        out[0, 8192 * c : 8192 * (c + 1), :] = res.results[c]["out"]
    return out


# revision 30
# speedup vs baseline: 1.8309x; 1.2935x over previous
"""OCAB (overlapping cross-attention block) Trainium2 Bass kernel, v2.

Full inputs in, full outputs out; internally shards the B*nW window axis
across 8 NeuronCores (each core owns 2 window-rows = 32 image rows, with a
4-row halo for the overlapping k/v windows).

v2 layout changes vs v1 (which was PE-bound at isolated-matmul latency and
Sync-bound on ~29 tiny SBUF->SBUF v-gather DMAs per window):
  - V slab stored column-major in 3 overlapping 128-col blocks
    (slab cols 0-127 / 112-239 / 136-263) so every 24-col window slice
    lives in one block at a contiguous partition range; the per-window
    v-gather is 5 DMAs (one per 5-row key chunk), each [24p, rn*192]
    contiguous-source -> [kc, 192] key-major tile.
  - Window key order is column-major within a chunk (key = rn*cl + rl),
    consistently in the k^T chunk tiles and the v tiles.
  - S^T PSUM is one 3-bank [kc, 1536] tile (head col pairs share banks
    only when they share a PE row group, except h2/h3 which Tile
    serializes); exp is a single ACT instruction per chunk.
  - attn@V runs chunk-major so PE accumulates av(ch) while ACT computes
    exp(ch+1); S^T(ch+1) follows once the st bank frees.
  - x loads batched 4 token-chunks per DMA; LN rstd via Sqrt(bias=eps) +
    reciprocal_approx_fast; normalize split across DVE and GpSimd.
"""

import os
import sys
from contextlib import ExitStack

import numpy as np
import ml_dtypes

for _p in ("/opt/trn_rl_repo", "/root/.axon_site/_ro/trn_rl_repo"):
    if os.path.isdir(_p) and _p not in sys.path:
        sys.path.append(_p)

import concourse.bass as bass
import concourse.tile as tile
from concourse import bacc, mybir
from concourse.bass_utils import run_bass_kernel_spmd

BF16 = mybir.dt.bfloat16
F32 = mybir.dt.float32
F32R = mybir.dt.float32r
bfnp = ml_dtypes.bfloat16

# ---- problem constants (hardcoded per contract) ----
C = 180
NH = 6
HD = 30
WS = 16
OWS = 24
PADW = 4
H = W = 256
EPS = 1e-5
NCORES = 8

# ---- per-core slab geometry ----
RS = 40          # slab image rows (32 + 2*4 halo)
CS = 264         # slab image cols (256 + 2*4 zero pad)
TS_REAL = RS * CS          # 10560 real slab tokens
TCH = 84                   # token chunks of 128
T = TCH * 128              # 10752 padded slab tokens
NG = 21                    # 512-token groups
NWIN = 32                  # windows per core (2 window-rows x 16)
CHUNK_ROWS = [5, 5, 5, 5, 4]
CHUNK_KC = [r * OWS for r in CHUNK_ROWS]   # 120,120,120,120,96
# head -> column in both the 3-bank S^T psum tile and the es sbuf tile.
# Heads sharing a psum bank MUST share a PE row group (the sub-array then
# serializes their writes; different row groups would write one bank
# concurrently = HW error). h3 therefore lives in the q45/k45 slabs at
# rows 64-95: bank pairs (h0,h4)@rows0, (h1,h5)@rows32, (h2,h3)@rows64.
HCOL = {0: 0, 4: 256, 1: 512, 5: 768, 2: 1024, 3: 1280}
SROW = {0: 0, 1: 32, 2: 64, 3: 64, 4: 0, 5: 32}   # PE row group per head
SORDER = [0, 1, 2, 4, 5, 3]
# v slab column blocks: block cg holds slab cols VCG_BASE[cg]..+127,
# partition = col - VCG_BASE[cg]; window wc reads block WC_CG[wc].
VCG_BASE = [0, 112, 136]
WC_CG = [0] * 7 + [1] * 7 + [2] * 2
VS_COLS = 3 * RS * 192     # cg-major block layout: block = 40*cg + r

LAST_RESULTS = None
_CACHED = None


def _build_program():
    nc = bacc.Bacc("TRN2", target_bir_lowering=False)

    xs_d = nc.declare_dram_parameter("xs", [T, C], F32, isOutput=False)
    xr_d = nc.declare_dram_parameter("xr", [8192, C], F32, isOutput=False)
    wqk_d = nc.declare_dram_parameter("wqk", [181, 512], BF16, isOutput=False)
    wv_d = nc.declare_dram_parameter("wv", [181, 192], BF16, isOutput=False)
    wp_d = nc.declare_dram_parameter("wp", [192, C], BF16, isOutput=False)
    id_d = nc.declare_dram_parameter("ident", [128, 128], BF16, isOutput=False)
    e128_d = nc.declare_dram_parameter("e128", [128, 128], F32R, isOutput=False)
    e64_d = nc.declare_dram_parameter("e64", [64, 128], F32R, isOutput=False)
    ones_d = nc.declare_dram_parameter("ones", [1, T], BF16, isOutput=False)
    out_d = nc.declare_dram_parameter("out", [8192, C], F32, isOutput=True)

    with ExitStack() as ctx:
        tc = ctx.enter_context(tile.TileContext(nc))

        # ---- persistent slabs (live through both phases) ----
        keep = ctx.enter_context(tc.tile_pool(name="keep", bufs=1))
        QS = [
            keep.tile([128, T], BF16, tag=f"qs{i}", name=f"qs{i}") for i in range(4)
        ]
        VS = keep.tile([128, VS_COLS], BF16, tag="vs")

        WQK0 = keep.tile([128, 512], BF16, tag="wqk0")
        WQK1 = keep.tile([53, 512], BF16, tag="wqk1")
        WV0 = keep.tile([128, 192], BF16, tag="wv0")
        WV1 = keep.tile([53, 192], BF16, tag="wv1")
        WP0 = keep.tile([128, C], BF16, tag="wp0")
        WP1 = keep.tile([64, C], BF16, tag="wp1")
        IDT = keep.tile([128, 128], BF16, tag="id")
        E128 = keep.tile([128, 128], F32R, tag="e128")
        E64 = keep.tile([64, 128], F32R, tag="e64")
        EPST = keep.tile([128, 1], F32, tag="epst")
        nc.vector.memset(EPST[:], EPS)

        nc.sync.dma_start(WQK0[:], wqk_d[0:128, :])
        nc.sync.dma_start(WQK1[:], wqk_d[128:181, :])
        nc.sync.dma_start(WV0[:], wv_d[0:128, :])
        nc.sync.dma_start(WV1[:], wv_d[128:181, :])
        nc.sync.dma_start(WP0[:], wp_d[0:128, :])
        nc.sync.dma_start(WP1[:], wp_d[128:192, :])
        nc.sync.dma_start(IDT[:], id_d[:, :])
        nc.sync.dma_start(E128[:], e128_d[:, :])
        nc.sync.dma_start(E64[:], e64_d[:, :])

        # ================= phase 1+2: LN, transpose, projections ============
        with ExitStack() as pctx:
            ph2 = pctx.enter_context(tc.tile_pool(name="ph2", bufs=1))
            XT0 = ph2.tile([128, T], BF16, tag="xt0")     # xn^T channels 0-127
            XT1 = ph2.tile([53, T], BF16, tag="xt1")      # ch 128-179 + ones row
            nc.sync.dma_start(XT1[52:53, :], ones_d[:, :])

            p_x = pctx.enter_context(tc.tile_pool(name="p_x", bufs=3))
            p_sm = pctx.enter_context(tc.tile_pool(name="p_sm", bufs=8))
            p_ps = pctx.enter_context(tc.tile_pool(name="p_ps", bufs=2, space="PSUM"))

            for g in range(NG):            # 21 groups of 4 token chunks
                xt4 = p_x.tile([128, 720], F32, tag="x")
                nc.sync.dma_start(
                    xt4[:],
                    xs_d[512 * g : 512 * (g + 1), :]
                    .rearrange("(j p) c -> j p c", p=128)
                    .transpose([1, 0, 2]),
                )
                tp = p_ps.tile([128, 512], BF16, tag="tp")
                tp2 = p_ps.tile([52, 512], BF16, tag="tp2")
                for j in range(4):
                    xsl = xt4[:, 180 * j : 180 * (j + 1)]
                    stats = p_sm.tile([128, 6], F32, tag="st")
                    aggr = p_sm.tile([128, 2], F32, tag="ag")
                    nc.vector.bn_stats(stats[:], xsl)
                    nc.vector.bn_aggr(aggr[:], stats[:])
                    sd = p_sm.tile([128, 1], F32, tag="sd")
                    nc.scalar.activation(
                        sd[:], aggr[:, 1:2], mybir.ActivationFunctionType.Sqrt,
                        bias=EPST[:],
                    )
                    rstd = p_sm.tile([128, 1], F32, tag="rstd")
                    nc.vector.reciprocal_approx_fast(rstd[:], sd[:])
                    nmr = p_sm.tile([128, 1], F32, tag="nmr")
                    nc.vector.tensor_scalar(
                        nmr[:],
                        aggr[:, 0:1],
                        rstd[:],
                        -1.0,
                        op0=mybir.AluOpType.mult,
                        op1=mybir.AluOpType.mult,
                    )
                    # normalize on ScalarE's free affine: xn = 1.0*(x*rstd - mu*rstd)
                    xn = p_x.tile([128, C], BF16, tag="xn")
                    nc.scalar.activation(
                        xn[:],
                        xsl,
                        mybir.ActivationFunctionType.Identity,
                        bias=nmr[:],
                        scale=rstd[:],
                    )
                    nc.tensor.transpose(
                        tp[:, 128 * j : 128 * (j + 1)], xn[:, 0:128], IDT[:]
                    )
                    nc.tensor.transpose(
                        tp2[:, 128 * j : 128 * (j + 1)], xn[:, 128:180], IDT[:]
                    )
                nc.vector.tensor_copy(XT0[:, 512 * g : 512 * (g + 1)], tp[:])
                nc.vector.tensor_copy(XT1[0:52, 512 * g : 512 * (g + 1)], tp2[:])

            # q^T / k^T projections: 4 M-chunks (q03, q45, k03, k45)
            for mc in range(4):
                for ng in range(NG):
                    qp = p_ps.tile([128, 512], F32, tag="mm")
                    nc.tensor.matmul(
                        qp[:],
                        WQK0[:, 128 * mc : 128 * (mc + 1)],
                        XT0[:, 512 * ng : 512 * (ng + 1)],
                        start=True,
                        stop=False,
                    )
                    nc.tensor.matmul(
                        qp[:],
                        WQK1[:, 128 * mc : 128 * (mc + 1)],
                        XT1[:, 512 * ng : 512 * (ng + 1)],
                        start=False,
                        stop=True,
                    )
                    nc.vector.tensor_copy(
                        QS[mc][:, 512 * ng : 512 * (ng + 1)], qp[:]
                    )

            # v projection into column-major blocks: block b = 40*cg + r holds
            # v for slab cols VCG_BASE[cg]..+127 of image row r (col on
            # partitions), two blocks per psum tile.
            for i in range(60):
                vp = p_ps.tile([128, 384], F32, tag="vp")
                for j in range(2):
                    b = 2 * i + j
                    cg, r = divmod(b, RS)
                    tok0 = CS * r + VCG_BASE[cg]
                    nc.tensor.matmul(
                        vp[:, 192 * j : 192 * (j + 1)],
                        XT0[:, tok0 : tok0 + 128],
                        WV0[:],
                        start=True,
                        stop=False,
                    )
                    nc.tensor.matmul(
                        vp[:, 192 * j : 192 * (j + 1)],
                        XT1[:, tok0 : tok0 + 128],
                        WV1[:],
                        start=False,
                        stop=True,
                    )
                nc.vector.tensor_copy(VS[:, 384 * i : 384 * (i + 1)], vp[:])

        # ================= phase 3: windowed attention =======================
        with ExitStack() as actx:
            a_st = actx.enter_context(tc.tile_pool(name="a_st", bufs=2, space="PSUM"))
            a_av = actx.enter_context(tc.tile_pool(name="a_av", bufs=1, space="PSUM"))
            a_ep = actx.enter_context(tc.tile_pool(name="a_ep", bufs=1, space="PSUM"))
            a_es = actx.enter_context(tc.tile_pool(name="a_es", bufs=10))
            a_kv = actx.enter_context(tc.tile_pool(name="a_kv", bufs=12))
            a_r = actx.enter_context(tc.tile_pool(name="a_r", bufs=1))
            a_sb = actx.enter_context(tc.tile_pool(name="a_sb", bufs=2))

            qs_pat = [
                QS[i][:, 0:TS_REAL].rearrange("p (r c) -> p r c", c=CS)
                for i in range(4)
            ]
            # head-major av jobs (h4/h5 duplicated to fill psum col groups)
            av_jobs = [(h, 32 * (h % 4), 256 * (h // 4)) for h in range(NH)]
            av_jobs += [(h, 64 + 32 * (h - 4), 256) for h in (4, 5)]

            xr_pat = xr_d[:, :].rearrange("(r c) d -> r c d", c=W)
            out_pat = out_d[:, :].rearrange("(r c) d -> r c d", c=W)

            def emit_av_jobs(pw, jlist):
                for ji in jlist:
                    h, colp, colf = av_jobs[ji]
                    for ch in range(5):
                        kc = CHUNK_KC[ch]
                        nc.tensor.matmul(
                            pw["av"][colp : colp + 32, colf : colf + 256],
                            pw["vw"][ch][0:kc, 32 * h : 32 * h + 32],
                            pw["es"][ch][0:kc, HCOL[h] : HCOL[h] + 256],
                            start=(ch == 0),
                            stop=(ch == 4),
                            tile_position=(0, colp),
                        )

            def emit_tail(pw):
                # softmax normalize + project + residual + store for window pw
                pwrl, pc0, pword = pw["wrl"], pw["c0"], pw["w"]
                av = pw["av"]
                rsb = a_r.tile([128, 512], F32R, tag="rsb")
                nc.vector.tensor_copy(rsb[:], av[:])
                ex = a_ep.tile([128, 512], F32, tag="ep", name=f"ex{pword}")
                nc.tensor.matmul(
                    ex[:, 0:256], E128[:], rsb[:, 0:256], start=True, stop=True
                )
                nc.tensor.matmul(
                    ex[:, 256:512], E64[:], rsb[0:64, 256:512], start=True, stop=True
                )
                ex_sb = a_r.tile([128, 512], F32, tag="exsb")
                nc.vector.reciprocal_approx_fast(ex_sb[:], ex[:])
                att = a_sb.tile([128, 512], BF16, tag="att")
                nc.vector.tensor_tensor(
                    att[:], av[:], ex_sb[:], op=mybir.AluOpType.mult
                )
                pp = a_ep.tile([128, 360], F32, tag="ep", name=f"pp{pword}")
                for qc in range(2):
                    nc.tensor.matmul(
                        pp[:, 180 * qc : 180 * qc + 180],
                        att[:, 128 * qc : 128 * (qc + 1)],
                        WP0[:],
                        start=True,
                        stop=False,
                    )
                    nc.tensor.matmul(
                        pp[:, 180 * qc : 180 * qc + 180],
                        att[0:64, 256 + 128 * qc : 256 + 128 * (qc + 1)],
                        WP1[:],
                        start=False,
                        stop=True,
                    )
                xres = a_sb.tile([128, 360], F32, tag="xres")
                ot = a_sb.tile([128, 360], F32, tag="ot")
                for qc in range(2):
                    rq = WS * pwrl + 8 * qc
                    nc.sync.dma_start(
                        xres[:, 180 * qc : 180 * qc + 180],
                        xr_pat[rq : rq + 8, pc0 : pc0 + WS, :],
                    )
                nc.vector.tensor_tensor(
                    ot[:], pp[:], xres[:], op=mybir.AluOpType.add
                )
                for qc in range(2):
                    rq = WS * pwrl + 8 * qc
                    nc.sync.dma_start(
                        out_pat[rq : rq + 8, pc0 : pc0 + WS, :],
                        ot[:, 180 * qc : 180 * qc + 180],
                    )

            stage = int(os.environ.get("KSTAGE", "9"))
            nwin = NWIN if stage >= 1 else 0
            prev = None
            for w in range(nwin):
                wrl, wc = w // 16, w % 16
                r0, c0 = WS * wrl, WS * wc
                cg = WC_CG[wc]
                pbase = c0 - VCG_BASE[cg]

                # ---- per-chunk v gather + k^T chunk tiles (keys row-major:
                # key = 24*rl + cl). DMA partition dims must MATCH on both
                # sides (mismatched counts wedge the DGE), so gather one slab
                # row per DMA: [24p, 192] -> [24p, 192], no boundary splits
                # thanks to the column-block VS layout.
                vw, kw0c, kw1c = [], [], []
                for ch in range(5):
                    rn = CHUNK_ROWS[ch]
                    kc = CHUNK_KC[ch]
                    R0 = r0 + 5 * ch
                    v = a_kv.tile([128, 192], BF16, tag="vw", name=f"vw{w}_{ch}")
                    for rl in range(rn):
                        blk = (RS * cg + R0 + rl) * 192
                        eng = nc.sync if (ch + rl) % 2 == 0 else nc.gpsimd
                        eng.dma_start(
                            v[24 * rl : 24 * rl + 24, :],
                            VS[pbase : pbase + 24, blk : blk + 192],
                        )
                    vw.append(v)
                    k0 = a_kv.tile([128, 120], BF16, tag="kw0", name=f"k0_{w}_{ch}")
                    nc.vector.tensor_copy(
                        k0[:, 0:kc].rearrange("p (rl cl) -> p rl cl", cl=24),
                        qs_pat[2][:, R0 : R0 + rn, c0 : c0 + 24],
                    )
                    kw0c.append(k0)
                    k1 = a_kv.tile([96, 120], BF16, tag="kw1", name=f"k1_{w}_{ch}")
                    nc.vector.tensor_copy(
                        k1[:, 0:kc].rearrange("p (rl cl) -> p rl cl", cl=24),
                        qs_pat[3][0:96, R0 : R0 + rn, c0 : c0 + 24],
                    )
                    kw1c.append(k1)

                if stage < 2:
                    continue

                # av for the PREVIOUS window accumulates interleaved between
                # this window's S^T chunks: ~2 av jobs fill the PE while
                # ScalarE runs each chunk's exp.
                if prev is not None:
                    prev["av"] = a_av.tile(
                        [128, 512], F32, tag="av", name=f"av{prev['w']}"
                    )

                es_list = []
                for ch in range(5):
                    kc = CHUNK_KC[ch]
                    st = a_st.tile([128, 1536], F32, tag="st", name=f"st{w}_{ch}")
                    for h in SORDER:
                        hr = SROW[h]
                        ktile = kw0c[ch] if h < 3 else kw1c[ch]
                        qtile = qs_pat[0] if h < 3 else qs_pat[1]
                        nc.tensor.matmul(
                            st[0:kc, HCOL[h] : HCOL[h] + 256],
                            ktile[hr : hr + 32, 0:kc],
                            qtile[
                                hr : hr + 32,
                                PADW + r0 : PADW + r0 + WS,
                                PADW + c0 : PADW + c0 + WS,
                            ],
                            start=True,
                            stop=True,
                            tile_position=(hr, 0),
                        )
                    es = a_es.tile([128, 1536], BF16, tag="es", name=f"es{w}_{ch}")
                    nc.scalar.activation(
                        es[0:kc, :], st[0:kc, :], mybir.ActivationFunctionType.Exp
                    )
                    es_list.append(es)
                    if prev is not None and ch < 4:
                        emit_av_jobs(prev, [2 * ch, 2 * ch + 1])

                if prev is not None:
                    emit_tail(prev)
                prev = {"w": w, "wrl": wrl, "c0": c0, "es": es_list, "vw": vw}

            if prev is not None and stage >= 2:
                prev["av"] = a_av.tile([128, 512], F32, tag="av", name=f"av{prev['w']}")
                emit_av_jobs(prev, range(len(av_jobs)))
                emit_tail(prev)

    nc.compile()
    return nc


def _prep_host(inputs):
    x = np.ascontiguousarray(inputs["x"], dtype=np.float32)[0]  # [65536, 180]
    norm_w = np.asarray(inputs["norm_w"], np.float32)
    norm_b = np.asarray(inputs["norm_b"], np.float32)
    q_w = np.asarray(inputs["q_w"], np.float32)
    q_b = np.asarray(inputs["q_b"], np.float32)
    kv_w = np.asarray(inputs["kv_w"], np.float32)
    kv_b = np.asarray(inputs["kv_b"], np.float32)
    proj_w = np.asarray(inputs["proj_w"], np.float32)
    proj_b = np.asarray(inputs["proj_b"], np.float32)

    scale = HD ** -0.5
    Wq = norm_w[:, None] * q_w * scale
    bq = (norm_b @ q_w + q_b) * scale
    Wk = norm_w[:, None] * kv_w[:, :C]
    bk = norm_b @ kv_w[:, :C] + kv_b[:C]
    Wv = norm_w[:, None] * kv_w[:, C:]
    bv = norm_b @ kv_w[:, C:] + kv_b[C:]

    # wqk [181, 512]: 4 M-chunks (q h0-2 | q h4,h5,h3 | k h0-2 | k h4,h5,h3)
    # h3 rides the second chunk at rows 64-95 so its psum-bank partner h2
    # shares PE row group 64 (see SROW in _build_program).
    wqk = np.zeros((181, 512), np.float32)
    for h in range(NH):
        if h < 3:
            col = 32 * h
        else:
            col = 128 + {3: 64, 4: 0, 5: 32}[h]
        wqk[:C, col : col + HD] = Wq[:, HD * h : HD * (h + 1)]
        wqk[C, col : col + HD] = bq[HD * h : HD * (h + 1)]
        colk = 256 + col
        wqk[:C, colk : colk + HD] = Wk[:, HD * h : HD * (h + 1)]
        wqk[C, colk : colk + HD] = bk[HD * h : HD * (h + 1)]

    # wv [181, 192]: 32-col head blocks; cols 30/31 of each block = ones
    wv = np.zeros((181, 192), np.float32)
    for h in range(NH):
        col = 32 * h
        wv[:C, col : col + HD] = Wv[:, HD * h : HD * (h + 1)]
        wv[C, col : col + HD] = bv[HD * h : HD * (h + 1)]
        wv[C, col + 30] = 1.0
        wv[C, col + 31] = 1.0

    # wp [192, 180]: head-padded proj rows
    wp = np.zeros((192, C), np.float32)
    for h in range(NH):
        row = 32 * (h % 4) if h < 4 else 128 + 32 * (h - 4)
        wp[row : row + HD, :] = proj_w[HD * h : HD * (h + 1), :]

    ident = np.eye(128, dtype=bfnp)
    e128 = np.zeros((128, 128), np.float32)
    for j in range(4):
        e128[32 * j + 30, 32 * j : 32 * j + 32] = 1.0
    e64 = np.zeros((64, 128), np.float32)
    for j in range(2):
        e64[32 * j + 30, 32 * j : 32 * j + 32] = 1.0
        e64[32 * j + 30, 64 + 32 * j : 64 + 32 * j + 32] = 1.0

    # per-core slabs
    xg = x.reshape(H, W, C)
    xpad = np.zeros((H + 2 * PADW, CS, C), np.float32)
    xpad[PADW : PADW + H, PADW : PADW + W, :] = xg
    xres_full = x + proj_b

    in_maps = []
    for c in range(NCORES):
        slab = np.zeros((T, C), np.float32)
        slab[:TS_REAL] = xpad[32 * c : 32 * c + RS].reshape(TS_REAL, C)
        xr = np.ascontiguousarray(
            xres_full[8192 * c : 8192 * (c + 1)], dtype=np.float32
        )
        in_maps.append(
            {
                "xs": slab,
                "xr": xr,
                "wqk": wqk.astype(bfnp),
                "wv": wv.astype(bfnp),
                "wp": wp.astype(bfnp),
                "ident": ident,
                "e128": e128,
                "e64": e64,
                "ones": np.ones((1, T), bfnp),
            }
        )
    return in_maps


def kernel(**inputs):
    global _CACHED, LAST_RESULTS
    if _CACHED is None:
        _CACHED = _build_program()
    nc = _CACHED
    in_maps = _prep_host(inputs)
    res = run_bass_kernel_spmd(
        nc,
        in_maps,
        list(range(NCORES)),
        trace=False,
    )
    LAST_RESULTS = res
    out = np.empty((1, H * W, C), np.float32)
    for c in range(NCORES):
        out[0, 8192 * c : 8192 * (c + 1), :] = res.results[c]["out"]
    return out
